# revision 13
# baseline (speedup 1.0000x reference)
"""Trainium2 Bass kernel for nn_Decoder_14894946583396 (dense_mlp).

Reference computation:
    sized = broadcast(representation[B,1,R] -> [B,S,R])   (ones @ rep)
    h     = relu(sized @ W1^T + b1)                       [B,S,HID]
    out   = h @ W2^T + b2                                 [B,S,OUT]

Because every position s within batch b receives the identical input row
representation[b], the MLP output row is identical for all S positions:
    row[b] = relu(rep[b] @ W1^T + b1) @ W2^T + b2         [B,OUT]
    out[b, s, :] = row[b]  for all s

The kernel computes the tiny per-batch MLP on the TensorEngine and
broadcast-writes each row across S with wide SBUF->DRAM DMAs.
Data-parallel across 8 NeuronCores: 4 batches per core, replicated
weights.

Everything on the device runs in bf16 (f32 PSUM accumulation): the
tolerance is 2e-2 and the measured end-to-end bf16 rounding error is
~4e-3.  This halves all three DMA phases vs an f32 version: weights
stream in as 2x1MiB, the PE matmuls stream 1 col/cycle, and the output
is stored as bf16 [BPC,S,OUT] (8 MiB/core) which the host upcasts to
f32 after the gather.

Device pipeline per core (timeline targets in parens):
  1. Input DMAs: xt (8 KiB x^T, scalar-ring HWDGE, lands first), then
     w1 -> w2 in FIFO order on the sync-ring HWDGE so w1 completes
     ~2.5 us before w2 — exactly when L1 needs it — with no SWDGE
     emission latency.  prow = {b1, ones, b2} rides SWDGE.  The 4x4
     transpose identity and the selector-broadcast blocks are memset
     on GpSimd instead of DMAed.
  2. Dummy matmuls on zeros bridge the PE from kernel start until w1
     lands, warming the HAM clock gate (1.2 -> 2.4 GHz) with no idle
     gap that would re-throttle it.
  3. L1: H[m,h] = x @ W1^T via a K=1 ones-matmul (bias, first so the
     accumulation ends on the last w1 chunk) plus 8 accumulating bf16
     matmuls with the tiny x^T chunk as stationary; relu on ScalarE
     (PSUM f32 -> SBUF bf16).
  4. H -> H^T via 4 PE transposes (stationary operand for L2).
  5. L2: Y[m,o] = H @ W2^T + b2, bias-first, 10 matmuls into 2 PSUM
     banks.
  6. Broadcast: K=4 selector matmul (lhsT = e_b outer ones) extracts
     row b of Y AND replicates it across all 128 output partitions in
     one PE op -> [128,512] PSUM tiles; one PSUM->SBUF bf16 copy per
     half, then log-doubling SBUF->SBUF replication copies (one writer
     engine per tile).
  7. 8 output DMAs of 1 MiB each (bf16) on the sync HWDGE ring.

Single-sync-wait discipline (this walrus rejects 2+ waits on any
instruction): every consumer sees at most one new semaphore because
earlier instructions with free wait slots pre-observe upcoming lanes
(warmup tail observes w1, L1 rc1 observes the GpSimd memsets, L1 rc7
observes w2) and vector clocks propagate transitively through the
copy chain, so the output DMAs can reuse input DMAHW lanes without
extra waits.  A chain of 1-wait SP nops before the TileContext exit
drain leaves the drain with nothing to wait on.
"""

import sys

import numpy as np

if "/opt/trn_rl_repo" not in sys.path:
    sys.path.insert(0, "/opt/trn_rl_repo")

import ml_dtypes

BF16 = np.dtype(ml_dtypes.bfloat16)

B, S, R = 32, 1024, 1024
HID, OUT = 512, 1024
N_CORES = 8
BPC = B // N_CORES  # batches per core

RC = R // 128  # layer-1 contraction chunks
HC = HID // 128  # layer-2 contraction chunks
OC = OUT // 512  # 512-wide output column chunks

# pk1a columns: [p, rc*BPC + m] = rep[m, rc*128+p], then a 4x4 identity,
# then 4 selector-broadcast blocks: [k, SELOFF + b*128 + m] = (k == b)
XTOFF = 0
I4OFF = XTOFF + RC * BPC
SELOFF = I4OFF + BPC
PK1AW = SELOFF + BPC * 128
# prow columns (single partition row)
B1OFF = 0
ONOFF = B1OFF + HID
B2OFF = ONOFF + 128
PROWW = B2OFF + OUT
# w1: [p, rc*HID + h] = W1[h, rc*128+p];  w2: [p, hc*OUT + o] = W2[o, hc*128+p]

N_COPIES = 4  # row copies along the free dim of each broadcast tile
S_PER_DMA = 128 * N_COPIES  # s-positions covered per output DMA
N_DMAS = S // S_PER_DMA  # output DMAs per batch
N_WARMUP = 8

_CACHED_NC = None


def _build_nc():
    import concourse.bass as bass
    import concourse.mybir as mybir
    from concourse.tile import TileContext, add_dep_helper

    bf16 = mybir.dt.bfloat16
    f32 = mybir.dt.float32
    relu = mybir.ActivationFunctionType.Relu
    fcopy = mybir.ActivationFunctionType.Copy
    nc = bass.Bass()

    pk1a = nc.dram_tensor("pk1a", [128, PK1AW], bf16, kind="ExternalInput")
    prow = nc.dram_tensor("prow", [1, PROWW], bf16, kind="ExternalInput")
    w1 = nc.dram_tensor("w1", [128, RC * HID], bf16, kind="ExternalInput")
    w2 = nc.dram_tensor("w2", [128, HC * OUT], bf16, kind="ExternalInput")
    out = nc.dram_tensor("out", [BPC, S, OUT], bf16, kind="ExternalOutput")

    with TileContext(nc) as tc:
        with (
            tc.tile_pool(name="const", bufs=1) as cpool,
            tc.tile_pool(name="psum_s", bufs=1, space="PSUM") as pp_s,
            tc.tile_pool(name="psum_y", bufs=2, space="PSUM") as pp_y,
            tc.tile_pool(name="psum_t", bufs=2, space="PSUM") as pp_t,
            tc.tile_pool(name="psum_bc", bufs=3, space="PSUM") as pp_bc,
        ):
            # sync-ring HWDGE FIFO: pk1a (tiny, unblocks the warmup
            # observer) then w1 then w2 — w1's data fully drains before w2
            # starts, landing ~2.5 us before L2 needs w2.
            p1a = cpool.tile([128, PK1AW], bf16, tag="pk1a")
            dma_p1a = nc.sync.dma_start(out=p1a[:, :], in_=pk1a[:, :])
            w1_sb = cpool.tile([128, RC * HID], bf16, tag="w1")
            dma_w1 = nc.sync.dma_start(out=w1_sb[:, :], in_=w1[:, :])
            w2_sb = cpool.tile([128, HC * OUT], bf16, tag="w2")
            dma_w2 = nc.sync.dma_start(out=w2_sb[:, :], in_=w2[:, :])
            # scalar-ring HWDGE: bias/ones row.  The scalar ring only drains
            # after the sync ring's queue, so prow lands last (~14.5 us) —
            # fine, the bias matmuls sit at the END of each accumulation.
            prow_sb = cpool.tile([1, PROWW], bf16, tag="prow")
            dma_prow = nc.scalar.dma_start(out=prow_sb[0:1, :], in_=prow[0:1, :])

            # ---- PE warmup on zeros; shares L1's PSUM tile (a slot handoff
            # would emit a non-elidable same-engine wait).  Bridges the PE
            # from kernel start until w1 lands so HAM never re-throttles. ---
            wm_sb = cpool.tile([128, 512], bf16, tag="wm")
            nc.vector.memset(wm_sb[:, :], 0.0)
            ph_full = pp_s.tile([128, HID], f32, tag="s")
            for k in range(N_WARMUP):
                wmm = nc.tensor.matmul(
                    ph_full[:, :],
                    lhsT=wm_sb[:, 0:128],
                    rhs=wm_sb[:, :],
                    start=True,
                    stop=True,
                )
                if k == 4:
                    # free wait slot on an early warmup (p1a's sem fires
                    # ~when this matmul runs): pre-observe p1a's lane so
                    # L1 rc0's only new wait is w1's lane
                    add_dep_helper(
                        wmm.ins, dma_p1a.ins, sync=True, reason="observe p1a"
                    )

            # ---- L1: H[m, h] = x @ W1^T + b1, relu -------------------------
            # bias last: prow rides the slow scalar ring and lands after w1
            ph = ph_full[0:BPC, :]
            for rc in range(RC):
                mm = nc.tensor.matmul(
                    ph[:, :],
                    lhsT=p1a[:, XTOFF + rc * BPC : XTOFF + (rc + 1) * BPC],
                    rhs=w1_sb[:, rc * HID : rc * HID + HID],
                    start=(rc == 0),
                    stop=False,
                )
            nc.tensor.matmul(
                ph[:, :],
                lhsT=prow_sb[0:1, ONOFF : ONOFF + BPC],
                rhs=prow_sb[0:1, B1OFF : B1OFF + HID],
                start=False,
                stop=True,
            )
            h_sb = cpool.tile([BPC, HID], bf16, tag="h")
            nc.scalar.activation(h_sb[:, :], ph[:, :], relu)

            # ---- H -> H^T (stationary operand for L2) ----------------------
            ht_sb = cpool.tile([128, HC * BPC], bf16, tag="ht")
            for hc in range(HC):
                pt = pp_t.tile([128, BPC], bf16, tag="t")
                tr = nc.tensor.transpose(
                    pt[:, :],
                    h_sb[0:BPC, hc * 128 : (hc + 1) * 128],
                    p1a[0:BPC, I4OFF : I4OFF + BPC],
                )
                if hc == 1:
                    # free wait slot (h_sb wait covered by hc=0): pre-observe
                    # w2's lane for L2 off L1's critical path
                    add_dep_helper(tr.ins, dma_w2.ins, sync=True, reason="observe w2")
                nc.scalar.activation(
                    ht_sb[:, hc * BPC : (hc + 1) * BPC], pt[:, :], fcopy
                )

            # ---- L2: Y[m, o] = b2 + H @ W2^T -------------------------------
            # per-oc Y tiles so the broadcast of the first half can start
            # while the second half's matmuls still run
            y_halves = []
            for oc in range(OC):
                py = pp_y.tile([BPC, 512], f32, tag="y")
                for hc in range(HC):
                    nc.tensor.matmul(
                        py[:, :],
                        lhsT=ht_sb[:, hc * BPC : (hc + 1) * BPC],
                        rhs=w2_sb[:, hc * OUT + oc * 512 : hc * OUT + oc * 512 + 512],
                        start=(hc == 0),
                        stop=False,
                    )
                nc.tensor.matmul(
                    py[:, :],
                    lhsT=prow_sb[0:1, ONOFF : ONOFF + BPC],
                    rhs=prow_sb[0:1, B2OFF + oc * 512 : B2OFF + (oc + 1) * 512],
                    start=False,
                    stop=True,
                )
                yh = cpool.tile([BPC, 512], bf16, tag=f"yh{oc}")
                nc.vector.tensor_copy(yh[:, :], py[:, :])
                y_halves.append(yh)

            # ---- broadcast rows across partitions, replicate, store --------
            # A K=4 selector matmul (lhsT = e_b outer ones, GpSimd-memset)
            # extracts row b of Y AND replicates it across all 128 output
            # partitions in one PE op — both operands at base partition 0.
            # Writer-engine split keeps batch 0's chain unobstructed on DVE
            # (its tile gates the first output DMA): b0 -> DVE, b1/b3 -> ACT,
            # b2 -> DVE casts + GpSimd doublings (GpSimd's clock covers the
            # DVE casts transitively, so b2's DMA still sees one writer).
            out_dmas = []
            yts = []
            for b in range(BPC):
                yt = cpool.tile([128, N_COPIES * OUT], bf16, tag=f"yt{b}")
                cast_eng = "dve" if b % 2 == 0 else "act"
                dbl_eng = cast_eng
                if b == 2:
                    # order-only edge: keep b2's DVE work behind b0's last
                    # doubling so the scheduler cannot delay b0's tile (it
                    # gates the first output DMA)
                    first_of_b2 = True
                for oc in range(OC):
                    pb = pp_bc.tile([128, 512], f32, tag="bc")
                    mm = nc.tensor.matmul(
                        pb[:, :],
                        lhsT=p1a[0:BPC, SELOFF + b * 128 : SELOFF + (b + 1) * 128],
                        rhs=y_halves[oc][0:BPC, :],
                        start=True,
                        stop=True,
                    )
                    last_mm = mm
                    # PSUM -> SBUF once per oc half (f32 -> bf16 cast)...
                    dst = yt[:, oc * 512 : (oc + 1) * 512]
                    if cast_eng == "dve":
                        cp = nc.vector.tensor_copy(dst, pb[:, :])
                        if b == 2 and first_of_b2:
                            add_dep_helper(
                                cp.ins, last_dve.ins, sync=False,
                                reason="order b2 after b0 doublings",
                            )
                            first_of_b2 = False
                        last_dve = cp
                    else:
                        last_act = nc.scalar.activation(dst, pb[:, :], fcopy)
                # ...then replicate with log-doubling SBUF->SBUF copies
                ncur = OUT
                while ncur < N_COPIES * OUT:
                    dst = yt[:, ncur : 2 * ncur]
                    if dbl_eng == "dve":
                        last_dve = nc.vector.tensor_copy(dst, yt[:, 0:ncur])
                    else:
                        last_act = nc.scalar.activation(dst, yt[:, 0:ncur], fcopy)
                    ncur *= 2
                yts.append(yt)

            # Output DMAs: each writes S_PER_DMA consecutive s rows (all
            # identical).  Issue all j=0 DMAs before the j=1 DMAs: the 11
            # HWDGE DMAs round-robin over 8 DMAHW lanes, so the last three
            # wrap onto the input DMAs' lanes and carry a non-elidable
            # lane-reuse wait — they must be DMAs whose data wait is already
            # covered (the j=1 group, covered by their j=0 siblings).
            for j in range(N_DMAS):
                for b in range(BPC):
                    d = nc.sync.dma_start(
                        out=out[b, j * S_PER_DMA : (j + 1) * S_PER_DMA, :].rearrange(
                            "(p c) o -> p c o", c=N_COPIES
                        ),
                        in_=yts[b][:, :].rearrange("p (c o) -> p c o", o=OUT),
                    )
                    out_dmas.append(d)

            # The kernel-tail drain waits on every proc's final tick, but this
            # walrus allows at most ONE sync wait per instruction. Chain SP
            # nops, one dependency each, so SP's vector clock observes the
            # final tick of every DMA lane and engine before the drain.
            tail = out_dmas + [last_mm, last_act, last_dve]
            for d in tail:
                n = nc.sync.nop(nofuse=True)
                add_dep_helper(
                    n.ins, d.ins, sync=True, reason="observe final ticks pre-drain"
                )

    return nc


def _get_nc():
    global _CACHED_NC
    if _CACHED_NC is None:
        _CACHED_NC = _build_nc()
    return _CACHED_NC


def _prep_in_maps(representation, W1, b1, W2, b2):
    rep = np.asarray(representation, dtype=np.float32).reshape(B, R).astype(BF16)
    w1 = np.asarray(W1, dtype=np.float32).astype(BF16)
    w2 = np.asarray(W2, dtype=np.float32).astype(BF16)
    b1 = np.asarray(b1, dtype=np.float32).astype(BF16)
    b2 = np.asarray(b2, dtype=np.float32).astype(BF16)

    w1p = np.ascontiguousarray(
        w1.T.reshape(RC, 128, HID).transpose(1, 0, 2).reshape(128, RC * HID)
    )
    w2p = np.ascontiguousarray(
        w2.T.reshape(HC, 128, OUT).transpose(1, 0, 2).reshape(128, HC * OUT)
    )
    prow = np.zeros((1, PROWW), dtype=BF16)
    prow[0, B1OFF : B1OFF + HID] = b1
    prow[0, ONOFF : ONOFF + 128] = 1.0
    prow[0, B2OFF : B2OFF + OUT] = b2

    in_maps = []
    for c in range(N_CORES):
        xtc = rep[c * BPC : (c + 1) * BPC].T  # [R, BPC]
        pk1a = np.zeros((128, PK1AW), dtype=BF16)
        pk1a[:, XTOFF : XTOFF + RC * BPC] = (
            xtc.reshape(RC, 128, BPC).transpose(1, 0, 2).reshape(128, RC * BPC)
        )
        pk1a[0:BPC, I4OFF : I4OFF + BPC] = np.eye(BPC, dtype=BF16)
        for b in range(BPC):
            pk1a[b, SELOFF + b * 128 : SELOFF + (b + 1) * 128] = 1.0
        in_maps.append({"pk1a": pk1a, "prow": prow, "w1": w1p, "w2": w2p})
    return in_maps


def run_sharded(representation, W1, b1, W2, b2, **run_kwargs):
    """Compile+run on 8 cores; returns (full_output, BassKernelResults)."""
    from concourse.bass_utils import run_bass_kernel_spmd

    nc = _get_nc()
    in_maps = _prep_in_maps(representation, W1, b1, W2, b2)
    res = run_bass_kernel_spmd(nc, in_maps, core_ids=list(range(N_CORES)), **run_kwargs)
    full = np.concatenate(
        [np.asarray(r["out"]).astype(np.float32) for r in res.results], axis=0
    )
    return full, res


def kernel(representation, size_matrix=None, W1=None, b1=None, W2=None, b2=None):
    # size_matrix only contributes its shape in the reference (ones_like);
    # its values are unused.
    full, _ = run_sharded(representation, W1, b1, W2, b2)
    return full


# revision 14
# speedup vs baseline: 1.0215x; 1.0215x over previous
"""Trainium2 Bass kernel for nn_Decoder_14894946583396 (dense_mlp).

Reference computation:
    sized = broadcast(representation[B,1,R] -> [B,S,R])   (ones @ rep)
    h     = relu(sized @ W1^T + b1)                       [B,S,HID]
    out   = h @ W2^T + b2                                 [B,S,OUT]

Because every position s within batch b receives the identical input row
representation[b], the MLP output row is identical for all S positions:
    row[b] = relu(rep[b] @ W1^T + b1) @ W2^T + b2         [B,OUT]
    out[b, s, :] = row[b]  for all s

The kernel computes the tiny per-batch MLP on the TensorEngine and
broadcast-writes each row across S with wide SBUF->DRAM DMAs.
Data-parallel across 8 NeuronCores: 4 batches per core, replicated
weights.

Everything on the device runs in bf16 (f32 PSUM accumulation): the
tolerance is 2e-2 and the measured end-to-end bf16 rounding error is
~4e-3.  This halves all three DMA phases vs an f32 version and doubles
PE matmul throughput; the output is stored as bf16 [BPC,S,OUT]
(8 MiB/core) which the host upcasts to f32 after the gather.

Device pipeline per core:
  1. Four input DMAs in sync-ring HWDGE FIFO order:
       wA = {x^T chunks, 4x4 identity, selector blocks, W1^T rc0-3}
       wB = {W1^T rc4-7}
       prow = {b1, ones, b2}        (tiny)
       w2 = {W2^T}
     L1's first half starts as soon as wA's semaphore fires (~10.5 us),
     rc4-7 pipeline behind wB, and w2 lands exactly when L2 needs it.
     (The scalar HWDGE ring only drains after the sync ring's entire
     queue, and SWDGE's SBUF descriptor rings contend with SDMA engines
     7/15, so everything rides the sync ring.)
  2. Dummy matmuls on zeros bridge the PE from kernel start until wA
     lands, warming the HAM clock gate (1.2 -> 2.4 GHz); the last
     warmup's LDWEIGHTS carries the wA-lane wait so L1 itself never
     stalls cold.
  3. L1: H[m,h] = x @ W1^T via 8 accumulating bf16 matmuls with the
     tiny x^T chunk as stationary (cheap LDWEIGHTS), bias folded in as
     a K=1 ones-matmul at the end, relu on ScalarE (PSUM f32 -> SBUF
     bf16).
  4. H -> H^T via 4 PE transposes (stationary operand for L2).
  5. L2: Y[m,o] = H @ W2^T + b2, 10 matmuls into 2 PSUM banks.
  6. Broadcast: K=4 selector matmul (lhsT = e_b outer ones) extracts
     row b of Y AND replicates it across all 128 output partitions in
     one PE op -> [128,512] PSUM tiles; one PSUM->SBUF bf16 copy per
     half, then log-doubling SBUF->SBUF replication copies.  Writer
     engines: b0/b2 -> DVE (with an order-only edge keeping b2 behind
     b0, whose tile gates the first output DMA), b1/b3 -> ACT.
  7. 8 output DMAs of 1 MiB each (bf16) on the sync HWDGE ring: all
     j=0 DMAs first, then the j=1 group — the 12 HWDGE DMAs round-robin
     over 8 DMAHW lanes, so the last four wrap onto the input lanes and
     carry a non-elidable lane-reuse wait; they must be DMAs whose data
     wait is already covered (the j=1 group, covered by their j=0
     siblings).

Single-sync-wait discipline (this walrus rejects 2+ waits on any
instruction): every consumer sees at most one new semaphore because
instructions with free wait slots pre-observe upcoming lanes (last
warmup observes wA, L1 rc4 waits wB naturally, the L1 bias matmul
waits prow naturally, transpose hc=1 observes w2) and vector clocks
propagate transitively through the copy chain, so the output DMAs can
reuse input DMAHW lanes without extra waits.  A chain of 1-wait SP
nops before the TileContext exit drain leaves the drain with nothing
to wait on.
"""

import sys

import numpy as np

if "/opt/trn_rl_repo" not in sys.path:
    sys.path.insert(0, "/opt/trn_rl_repo")

import ml_dtypes

BF16 = np.dtype(ml_dtypes.bfloat16)

B, S, R = 32, 1024, 1024
HID, OUT = 512, 1024
N_CORES = 8
BPC = B // N_CORES  # batches per core

RC = R // 128  # layer-1 contraction chunks
RCA = RC // 2  # chunks carried by wA (rest in wB)
HC = HID // 128  # layer-2 contraction chunks
OC = OUT // 512  # 512-wide output column chunks

# wA columns: [p, rc*BPC + m] = rep[m, rc*128+p] (all RC chunks of x^T),
# then a 4x4 identity, then 4 selector-broadcast blocks
# [k, SELOFF + b*128 + m] = (k == b), then W1^T chunks rc0..RCA-1
# ([p, W1AOFF + rc*HID + h] = W1[h, rc*128+p]).
XTOFF = 0
I4OFF = XTOFF + RC * BPC
SELOFF = I4OFF + BPC
W1AOFF = SELOFF + BPC * 128
WAW = W1AOFF + RCA * HID
# wB columns: W1^T chunks RCA..RC-1
WBW = (RC - RCA) * HID
# prow columns (single partition row)
B1OFF = 0
ONOFF = B1OFF + HID
B2OFF = ONOFF + 128
PROWW = B2OFF + OUT
# w2: [p, hc*OUT + o] = W2[o, hc*128+p]

N_COPIES = 4  # row copies along the free dim of each broadcast tile
S_PER_DMA = 128 * N_COPIES  # s-positions covered per output DMA
N_DMAS = S // S_PER_DMA  # output DMAs per batch
N_WARMUP = 6

_CACHED_NC = None


def _build_nc():
    import concourse.bass as bass
    import concourse.mybir as mybir
    from concourse.tile import TileContext, add_dep_helper

    bf16 = mybir.dt.bfloat16
    f32 = mybir.dt.float32
    relu = mybir.ActivationFunctionType.Relu
    fcopy = mybir.ActivationFunctionType.Copy
    nc = bass.Bass()

    wA = nc.dram_tensor("wA", [128, WAW], bf16, kind="ExternalInput")
    wB = nc.dram_tensor("wB", [128, WBW], bf16, kind="ExternalInput")
    prow = nc.dram_tensor("prow", [1, PROWW], bf16, kind="ExternalInput")
    w2 = nc.dram_tensor("w2", [128, HC * OUT], bf16, kind="ExternalInput")
    out = nc.dram_tensor("out", [BPC, S, OUT], bf16, kind="ExternalOutput")

    with TileContext(nc) as tc:
        with (
            tc.tile_pool(name="const", bufs=1) as cpool,
            tc.tile_pool(name="psum_s", bufs=1, space="PSUM") as pp_s,
            tc.tile_pool(name="psum_y", bufs=2, space="PSUM") as pp_y,
            tc.tile_pool(name="psum_t", bufs=2, space="PSUM") as pp_t,
            tc.tile_pool(name="psum_bc", bufs=3, space="PSUM") as pp_bc,
        ):
            wA_sb = cpool.tile([128, WAW], bf16, tag="wA")
            dma_wA = nc.sync.dma_start(out=wA_sb[:, :], in_=wA[:, :])
            wB_sb = cpool.tile([128, WBW], bf16, tag="wB")
            dma_wB = nc.sync.dma_start(out=wB_sb[:, :], in_=wB[:, :])
            prow_sb = cpool.tile([1, PROWW], bf16, tag="prow")
            dma_prow = nc.sync.dma_start(out=prow_sb[0:1, :], in_=prow[0:1, :])
            w2_sb = cpool.tile([128, HC * OUT], bf16, tag="w2")
            dma_w2 = nc.sync.dma_start(out=w2_sb[:, :], in_=w2[:, :])

            # ---- PE warmup on zeros; shares L1's PSUM tile (a slot handoff
            # would emit a non-elidable same-engine wait).  Bridges the PE
            # from kernel start until wA's semaphore fires. ------------------
            wm_sb = cpool.tile([128, 512], bf16, tag="wm")
            nc.vector.memset(wm_sb[:, :], 0.0)
            ph_full = pp_s.tile([128, HID], f32, tag="s")
            for k in range(N_WARMUP):
                wmm = nc.tensor.matmul(
                    ph_full[:, :],
                    lhsT=wm_sb[:, 0:128],
                    rhs=wm_sb[:, :],
                    start=True,
                    stop=True,
                )
            # the last warmup's LDWEIGHTS carries the wA-lane wait: earlier
            # warmups keep the PE busy while it waits, and L1 never stalls
            add_dep_helper(wmm.ins, dma_wA.ins, sync=True, reason="observe wA")

            # ---- L1: H[m, h] = x @ W1^T + b1, relu -------------------------
            ph = ph_full[0:BPC, :]
            for rc in range(RC):
                w1_src = wA_sb if rc < RCA else wB_sb
                w1_off = W1AOFF + rc * HID if rc < RCA else (rc - RCA) * HID
                mm = nc.tensor.matmul(
                    ph[:, :],
                    lhsT=wA_sb[:, XTOFF + rc * BPC : XTOFF + (rc + 1) * BPC],
                    rhs=w1_src[:, w1_off : w1_off + HID],
                    start=(rc == 0),
                    stop=False,
                )
            # bias last: a K=1 ones-matmul (prow's lane wait sits here,
            # naturally, off the rc0-3 critical path)
            nc.tensor.matmul(
                ph[:, :],
                lhsT=prow_sb[0:1, ONOFF : ONOFF + BPC],
                rhs=prow_sb[0:1, B1OFF : B1OFF + HID],
                start=False,
                stop=True,
            )
            h_sb = cpool.tile([BPC, HID], bf16, tag="h")
            nc.scalar.activation(h_sb[:, :], ph[:, :], relu)

            # ---- H -> H^T (stationary operand for L2) ----------------------
            ht_sb = cpool.tile([128, HC * BPC], bf16, tag="ht")
            for hc in range(HC):
                pt = pp_t.tile([128, BPC], bf16, tag="t")
                tr = nc.tensor.transpose(
                    pt[:, :],
                    h_sb[0:BPC, hc * 128 : (hc + 1) * 128],
                    wA_sb[0:BPC, I4OFF : I4OFF + BPC],
                )
                if hc == 1:
                    # free wait slot (h_sb wait covered by hc=0, fresh PSUM
                    # slot): pre-observe w2's lane for L2 off the critical
                    # path
                    add_dep_helper(tr.ins, dma_w2.ins, sync=True, reason="observe w2")
                nc.scalar.activation(
                    ht_sb[:, hc * BPC : (hc + 1) * BPC], pt[:, :], fcopy
                )

            # ---- L2: Y[m, o] = H @ W2^T + b2 -------------------------------
            # per-oc Y tiles so the broadcast of the first half can start
            # while the second half's matmuls still run
            y_halves = []
            for oc in range(OC):
                py = pp_y.tile([BPC, 512], f32, tag="y")
                for hc in range(HC):
                    nc.tensor.matmul(
                        py[:, :],
                        lhsT=ht_sb[:, hc * BPC : (hc + 1) * BPC],
                        rhs=w2_sb[:, hc * OUT + oc * 512 : hc * OUT + oc * 512 + 512],
                        start=(hc == 0),
                        stop=False,
                    )
                nc.tensor.matmul(
                    py[:, :],
                    lhsT=prow_sb[0:1, ONOFF : ONOFF + BPC],
                    rhs=prow_sb[0:1, B2OFF + oc * 512 : B2OFF + (oc + 1) * 512],
                    start=False,
                    stop=True,
                )
                yh = cpool.tile([BPC, 512], bf16, tag=f"yh{oc}")
                nc.vector.tensor_copy(yh[:, :], py[:, :])
                y_halves.append(yh)

            # ---- broadcast rows across partitions, replicate, store --------
            # A K=4 selector matmul (lhsT = e_b outer ones, host-packed)
            # extracts row b of Y AND replicates it across all 128 output
            # partitions in one PE op — both operands at base partition 0.
            out_dmas = []
            yts = []
            for b in range(BPC):
                yt = cpool.tile([128, N_COPIES * OUT], bf16, tag=f"yt{b}")
                copy_eng = "dve" if b % 2 == 0 else "act"
                if b == 2:
                    first_of_b2 = True
                for oc in range(OC):
                    pb = pp_bc.tile([128, 512], f32, tag="bc")
                    mm = nc.tensor.matmul(
                        pb[:, :],
                        lhsT=wA_sb[0:BPC, SELOFF + b * 128 : SELOFF + (b + 1) * 128],
                        rhs=y_halves[oc][0:BPC, :],
                        start=True,
                        stop=True,
                    )
                    last_mm = mm
                    # PSUM -> SBUF once per oc half (f32 -> bf16 cast)...
                    dst = yt[:, oc * 512 : (oc + 1) * 512]
                    if copy_eng == "dve":
                        cp = nc.vector.tensor_copy(dst, pb[:, :])
                        if b == 2 and first_of_b2:
                            # order-only edge: keep b2's DVE work behind b0's
                            # last doubling (b0's tile gates the first DMA)
                            add_dep_helper(
                                cp.ins, last_dve.ins, sync=False,
                                reason="order b2 after b0 doublings",
                            )
                            first_of_b2 = False
                        last_dve = cp
                    else:
                        last_act = nc.scalar.activation(dst, pb[:, :], fcopy)
                # ...then replicate with log-doubling SBUF->SBUF copies
                ncur = OUT
                while ncur < N_COPIES * OUT:
                    dst = yt[:, ncur : 2 * ncur]
                    if copy_eng == "dve":
                        last_dve = nc.vector.tensor_copy(dst, yt[:, 0:ncur])
                    else:
                        last_act = nc.scalar.activation(dst, yt[:, 0:ncur], fcopy)
                    ncur *= 2
                yts.append(yt)

            # Output DMAs: each writes S_PER_DMA consecutive s rows (all
            # identical).  All j=0 DMAs first, then the j=1 group (see
            # docstring).
            for j in range(N_DMAS):
                for b in range(BPC):
                    d = nc.sync.dma_start(
                        out=out[b, j * S_PER_DMA : (j + 1) * S_PER_DMA, :].rearrange(
                            "(p c) o -> p c o", c=N_COPIES
                        ),
                        in_=yts[b][:, :].rearrange("p (c o) -> p c o", o=OUT),
                    )
                    out_dmas.append(d)

            # The kernel-tail drain waits on every proc's final tick, but this
            # walrus allows at most ONE sync wait per instruction. Chain SP
            # nops, one dependency each, so SP's vector clock observes the
            # final tick of every DMA lane and engine before the drain.
            tail = out_dmas + [last_mm, last_act, last_dve]
            for d in tail:
                n = nc.sync.nop(nofuse=True)
                add_dep_helper(
                    n.ins, d.ins, sync=True, reason="observe final ticks pre-drain"
                )

    return nc


def _get_nc():
    global _CACHED_NC
    if _CACHED_NC is None:
        _CACHED_NC = _build_nc()
    return _CACHED_NC


def _prep_in_maps(representation, W1, b1, W2, b2):
    rep = np.asarray(representation, dtype=np.float32).reshape(B, R).astype(BF16)
    w1 = np.asarray(W1, dtype=np.float32).astype(BF16)
    w2 = np.asarray(W2, dtype=np.float32).astype(BF16)
    b1 = np.asarray(b1, dtype=np.float32).astype(BF16)
    b2 = np.asarray(b2, dtype=np.float32).astype(BF16)

    w1p = np.ascontiguousarray(
        w1.T.reshape(RC, 128, HID).transpose(1, 0, 2).reshape(128, RC * HID)
    )
    w2p = np.ascontiguousarray(
        w2.T.reshape(HC, 128, OUT).transpose(1, 0, 2).reshape(128, HC * OUT)
    )
    prow = np.zeros((1, PROWW), dtype=BF16)
    prow[0, B1OFF : B1OFF + HID] = b1
    prow[0, ONOFF : ONOFF + 128] = 1.0
    prow[0, B2OFF : B2OFF + OUT] = b2
    wBp = np.ascontiguousarray(w1p[:, RCA * HID :])

    in_maps = []
    for c in range(N_CORES):
        xtc = rep[c * BPC : (c + 1) * BPC].T  # [R, BPC]
        wAp = np.zeros((128, WAW), dtype=BF16)
        wAp[:, XTOFF : XTOFF + RC * BPC] = (
            xtc.reshape(RC, 128, BPC).transpose(1, 0, 2).reshape(128, RC * BPC)
        )
        wAp[0:BPC, I4OFF : I4OFF + BPC] = np.eye(BPC, dtype=BF16)
        for b in range(BPC):
            wAp[b, SELOFF + b * 128 : SELOFF + (b + 1) * 128] = 1.0
        wAp[:, W1AOFF:] = w1p[:, : RCA * HID]
        in_maps.append({"wA": wAp, "wB": wBp, "prow": prow, "w2": w2p})
    return in_maps


def run_sharded(representation, W1, b1, W2, b2, **run_kwargs):
    """Compile+run on 8 cores; returns (full_output, BassKernelResults)."""
    from concourse.bass_utils import run_bass_kernel_spmd

    nc = _get_nc()
    in_maps = _prep_in_maps(representation, W1, b1, W2, b2)
    res = run_bass_kernel_spmd(nc, in_maps, core_ids=list(range(N_CORES)), **run_kwargs)
    full = np.concatenate(
        [np.asarray(r["out"]).astype(np.float32) for r in res.results], axis=0
    )
    return full, res


def kernel(representation, size_matrix=None, W1=None, b1=None, W2=None, b2=None):
    # size_matrix only contributes its shape in the reference (ones_like);
    # its values are unused.
    full, _ = run_sharded(representation, W1, b1, W2, b2)
    return full


# revision 15
# speedup vs baseline: 1.0247x; 1.0032x over previous
"""Trainium2 Bass kernel for nn_Decoder_14894946583396 (dense_mlp).

Reference computation:
    sized = broadcast(representation[B,1,R] -> [B,S,R])   (ones @ rep)
    h     = relu(sized @ W1^T + b1)                       [B,S,HID]
    out   = h @ W2^T + b2                                 [B,S,OUT]

Because every position s within batch b receives the identical input row
representation[b], the MLP output row is identical for all S positions:
    row[b] = relu(rep[b] @ W1^T + b1) @ W2^T + b2         [B,OUT]
    out[b, s, :] = row[b]  for all s

The kernel computes the tiny per-batch MLP on the TensorEngine and
broadcast-writes each row across S with wide SBUF->DRAM DMAs.
Data-parallel across 8 NeuronCores: 4 batches per core, replicated
weights.

Everything on the device runs in bf16 (f32 PSUM accumulation): the
tolerance is 2e-2 and the measured end-to-end bf16 rounding error is
~4e-3.  This halves all three DMA phases vs an f32 version and doubles
PE matmul throughput; the output is stored as bf16 [BPC,S,OUT]
(8 MiB/core) which the host upcasts to f32 after the gather.

Device pipeline per core:
  1. Four input DMAs in sync-ring HWDGE FIFO order:
       wA = {x^T chunks, 4x4 identity, selector blocks, W1^T rc0-3}
       wB = {W1^T rc4-7}
       prow = {b1, ones, b2}        (tiny)
       w2 = {W2^T}
     L1's first half starts as soon as wA's semaphore fires (~10.5 us),
     rc4-7 pipeline behind wB, and w2 lands exactly when L2 needs it.
     (The scalar HWDGE ring only drains after the sync ring's entire
     queue, and SWDGE's SBUF descriptor rings contend with SDMA engines
     7/15, so everything rides the sync ring.)
  2. Dummy matmuls on zeros bridge the PE from kernel start until wA
     lands, warming the HAM clock gate (1.2 -> 2.4 GHz); the last
     warmup's LDWEIGHTS carries the wA-lane wait so L1 itself never
     stalls cold.
  3. L1: H[m,h] = x @ W1^T via 8 accumulating bf16 matmuls with the
     tiny x^T chunk as stationary (cheap LDWEIGHTS), bias folded in as
     a K=1 ones-matmul at the end, relu on ScalarE (PSUM f32 -> SBUF
     bf16).
  4. H -> H^T via 4 PE transposes (stationary operand for L2).
  5. L2: Y[m,o] = H @ W2^T + b2, 10 matmuls into 2 PSUM banks.
  6. Broadcast: K=4 selector matmul (lhsT = e_b outer ones) extracts
     row b of Y AND replicates it across all 128 output partitions in
     one PE op -> [128,512] PSUM tiles; one PSUM->SBUF bf16 copy per
     half, then log-doubling SBUF->SBUF replication copies.  Writer
     engines: b0/b2 -> DVE (with an order-only edge keeping b2 behind
     b0, whose tile gates the first output DMA), b1/b3 -> ACT.
  7. 8 output DMAs of 1 MiB each (bf16) on the sync HWDGE ring: all
     j=0 DMAs first, then the j=1 group — the 12 HWDGE DMAs round-robin
     over 8 DMAHW lanes, so the last four wrap onto the input lanes and
     carry a non-elidable lane-reuse wait; they must be DMAs whose data
     wait is already covered (the j=1 group, covered by their j=0
     siblings).

Single-sync-wait discipline (this walrus rejects 2+ waits on any
instruction): every consumer sees at most one new semaphore because
instructions with free wait slots pre-observe upcoming lanes (last
warmup observes wA, L1 rc4 waits wB naturally, the L1 bias matmul
waits prow naturally, transpose hc=1 observes w2) and vector clocks
propagate transitively through the copy chain, so the output DMAs can
reuse input DMAHW lanes without extra waits.  A chain of 1-wait SP
nops before the TileContext exit drain leaves the drain with nothing
to wait on.
"""

import sys

import numpy as np

if "/opt/trn_rl_repo" not in sys.path:
    sys.path.insert(0, "/opt/trn_rl_repo")

import ml_dtypes

BF16 = np.dtype(ml_dtypes.bfloat16)

B, S, R = 32, 1024, 1024
HID, OUT = 512, 1024
N_CORES = 8
BPC = B // N_CORES  # batches per core

RC = R // 128  # layer-1 contraction chunks
RCA = RC // 2  # chunks carried by wA (rest in wB)
HC = HID // 128  # layer-2 contraction chunks
OC = OUT // 512  # 512-wide output column chunks

# wA columns: [p, rc*BPC + m] = rep[m, rc*128+p] (all RC chunks of x^T),
# then a 4x4 identity, then 4 selector-broadcast blocks
# [k, SELOFF + b*128 + m] = (k == b), then W1^T chunks rc0..RCA-1
# ([p, W1AOFF + rc*HID + h] = W1[h, rc*128+p]).
XTOFF = 0
I4OFF = XTOFF + RC * BPC
SELOFF = I4OFF + BPC
W1AOFF = SELOFF + BPC * 128
WAW = W1AOFF + RCA * HID
# wB columns: W1^T chunks RCA..RC-1
WBW = (RC - RCA) * HID
# prow columns (single partition row)
B1OFF = 0
ONOFF = B1OFF + HID
B2OFF = ONOFF + 128
PROWW = B2OFF + OUT
# w2: [p, hc*OUT + o] = W2[o, hc*128+p]

N_COPIES = 4  # row copies along the free dim of each broadcast tile
S_PER_DMA = 128 * N_COPIES  # s-positions covered per output DMA
N_DMAS = S // S_PER_DMA  # output DMAs per batch
N_WARMUP = 7

_CACHED_NC = None


def _build_nc():
    import concourse.bass as bass
    import concourse.mybir as mybir
    from concourse.tile import TileContext, add_dep_helper

    bf16 = mybir.dt.bfloat16
    f32 = mybir.dt.float32
    relu = mybir.ActivationFunctionType.Relu
    fcopy = mybir.ActivationFunctionType.Copy
    nc = bass.Bass()

    wA = nc.dram_tensor("wA", [128, WAW], bf16, kind="ExternalInput")
    wB = nc.dram_tensor("wB", [128, WBW], bf16, kind="ExternalInput")
    prow = nc.dram_tensor("prow", [1, PROWW], bf16, kind="ExternalInput")
    w2 = nc.dram_tensor("w2", [128, HC * OUT], bf16, kind="ExternalInput")
    out = nc.dram_tensor("out", [BPC, S, OUT], bf16, kind="ExternalOutput")

    with TileContext(nc) as tc:
        with (
            tc.tile_pool(name="const", bufs=1) as cpool,
            tc.tile_pool(name="psum_s", bufs=1, space="PSUM") as pp_s,
            tc.tile_pool(name="psum_y", bufs=2, space="PSUM") as pp_y,
            tc.tile_pool(name="psum_t", bufs=2, space="PSUM") as pp_t,
            tc.tile_pool(name="psum_bc", bufs=3, space="PSUM") as pp_bc,
        ):
            wA_sb = cpool.tile([128, WAW], bf16, tag="wA")
            dma_wA = nc.sync.dma_start(out=wA_sb[:, :], in_=wA[:, :])
            wB_sb = cpool.tile([128, WBW], bf16, tag="wB")
            dma_wB = nc.sync.dma_start(out=wB_sb[:, :], in_=wB[:, :])
            prow_sb = cpool.tile([1, PROWW], bf16, tag="prow")
            dma_prow = nc.sync.dma_start(out=prow_sb[0:1, :], in_=prow[0:1, :])
            w2_sb = cpool.tile([128, HC * OUT], bf16, tag="w2")
            dma_w2 = nc.sync.dma_start(out=w2_sb[:, :], in_=w2[:, :])

            # ---- PE warmup on zeros; shares L1's PSUM tile (a slot handoff
            # would emit a non-elidable same-engine wait).  Bridges the PE
            # from kernel start until wA's semaphore fires. ------------------
            wm_sb = cpool.tile([128, 512], bf16, tag="wm")
            nc.vector.memset(wm_sb[:, :], 0.0)
            ph_full = pp_s.tile([128, HID], f32, tag="s")
            for k in range(N_WARMUP):
                wmm = nc.tensor.matmul(
                    ph_full[:, :],
                    lhsT=wm_sb[:, 0:128],
                    rhs=wm_sb[:, :],
                    start=True,
                    stop=True,
                )
            # the last warmup's LDWEIGHTS carries the wA-lane wait: earlier
            # warmups keep the PE busy while it waits, and L1 never stalls
            add_dep_helper(wmm.ins, dma_wA.ins, sync=True, reason="observe wA")

            # ---- L1: H[m, h] = x @ W1^T + b1, relu -------------------------
            ph = ph_full[0:BPC, :]
            for rc in range(RC):
                w1_src = wA_sb if rc < RCA else wB_sb
                w1_off = W1AOFF + rc * HID if rc < RCA else (rc - RCA) * HID
                mm = nc.tensor.matmul(
                    ph[:, :],
                    lhsT=wA_sb[:, XTOFF + rc * BPC : XTOFF + (rc + 1) * BPC],
                    rhs=w1_src[:, w1_off : w1_off + HID],
                    start=(rc == 0),
                    stop=False,
                )
            # bias last: a K=1 ones-matmul (prow's lane wait sits here,
            # naturally, off the rc0-3 critical path)
            nc.tensor.matmul(
                ph[:, :],
                lhsT=prow_sb[0:1, ONOFF : ONOFF + BPC],
                rhs=prow_sb[0:1, B1OFF : B1OFF + HID],
                start=False,
                stop=True,
            )
            h_sb = cpool.tile([BPC, HID], bf16, tag="h")
            nc.scalar.activation(h_sb[:, :], ph[:, :], relu)

            # ---- H -> H^T (stationary operand for L2) ----------------------
            ht_sb = cpool.tile([128, HC * BPC], bf16, tag="ht")
            for hc in range(HC):
                pt = pp_t.tile([128, BPC], bf16, tag="t")
                tr = nc.tensor.transpose(
                    pt[:, :],
                    h_sb[0:BPC, hc * 128 : (hc + 1) * 128],
                    wA_sb[0:BPC, I4OFF : I4OFF + BPC],
                )
                if hc == 1:
                    # free wait slot (h_sb wait covered by hc=0, fresh PSUM
                    # slot): pre-observe w2's lane for L2 off the critical
                    # path
                    add_dep_helper(tr.ins, dma_w2.ins, sync=True, reason="observe w2")
                nc.scalar.activation(
                    ht_sb[:, hc * BPC : (hc + 1) * BPC], pt[:, :], fcopy
                )

            # ---- L2: Y[m, o] = H @ W2^T + b2 -------------------------------
            # per-oc Y tiles so the broadcast of the first half can start
            # while the second half's matmuls still run
            y_halves = []
            for oc in range(OC):
                py = pp_y.tile([BPC, 512], f32, tag="y")
                for hc in range(HC):
                    nc.tensor.matmul(
                        py[:, :],
                        lhsT=ht_sb[:, hc * BPC : (hc + 1) * BPC],
                        rhs=w2_sb[:, hc * OUT + oc * 512 : hc * OUT + oc * 512 + 512],
                        start=(hc == 0),
                        stop=False,
                    )
                nc.tensor.matmul(
                    py[:, :],
                    lhsT=prow_sb[0:1, ONOFF : ONOFF + BPC],
                    rhs=prow_sb[0:1, B2OFF + oc * 512 : B2OFF + (oc + 1) * 512],
                    start=False,
                    stop=True,
                )
                yh = cpool.tile([BPC, 512], bf16, tag=f"yh{oc}")
                nc.scalar.activation(yh[:, :], py[:, :], fcopy)
                y_halves.append(yh)

            # ---- broadcast rows across partitions, replicate, store --------
            # A K=4 selector matmul (lhsT = e_b outer ones, host-packed)
            # extracts row b of Y AND replicates it across all 128 output
            # partitions in one PE op — both operands at base partition 0.
            out_dmas = []
            yts = []
            for b in range(BPC):
                yt = cpool.tile([128, N_COPIES * OUT], bf16, tag=f"yt{b}")
                copy_eng = "dve" if b % 2 == 0 else "act"
                if b == 2:
                    first_of_b2 = True
                for oc in range(OC):
                    pb = pp_bc.tile([128, 512], f32, tag="bc")
                    mm = nc.tensor.matmul(
                        pb[:, :],
                        lhsT=wA_sb[0:BPC, SELOFF + b * 128 : SELOFF + (b + 1) * 128],
                        rhs=y_halves[oc][0:BPC, :],
                        start=True,
                        stop=True,
                    )
                    last_mm = mm
                    # PSUM -> SBUF once per oc half (f32 -> bf16 cast)...
                    dst = yt[:, oc * 512 : (oc + 1) * 512]
                    if copy_eng == "dve":
                        cp = nc.vector.tensor_copy(dst, pb[:, :])
                        if b == 2 and first_of_b2:
                            # order-only edge: keep b2's DVE work behind b0's
                            # last doubling (b0's tile gates the first DMA)
                            add_dep_helper(
                                cp.ins, last_dve.ins, sync=False,
                                reason="order b2 after b0 doublings",
                            )
                            first_of_b2 = False
                        last_dve = cp
                    else:
                        last_act = nc.scalar.activation(dst, pb[:, :], fcopy)
                # ...then replicate with log-doubling SBUF->SBUF copies
                ncur = OUT
                while ncur < N_COPIES * OUT:
                    dst = yt[:, ncur : 2 * ncur]
                    if copy_eng == "dve":
                        last_dve = nc.vector.tensor_copy(dst, yt[:, 0:ncur])
                    else:
                        last_act = nc.scalar.activation(dst, yt[:, 0:ncur], fcopy)
                    ncur *= 2
                yts.append(yt)

            # Output DMAs: each writes S_PER_DMA consecutive s rows (all
            # identical).  All j=0 DMAs first, then the j=1 group (see
            # docstring).
            for j in range(N_DMAS):
                for b in range(BPC):
                    d = nc.sync.dma_start(
                        out=out[b, j * S_PER_DMA : (j + 1) * S_PER_DMA, :].rearrange(
                            "(p c) o -> p c o", c=N_COPIES
                        ),
                        in_=yts[b][:, :].rearrange("p (c o) -> p c o", o=OUT),
                    )
                    out_dmas.append(d)

            # The kernel-tail drain waits on every proc's final tick, but this
            # walrus allows at most ONE sync wait per instruction. Chain SP
            # nops, one dependency each, so SP's vector clock observes the
            # final tick of every DMA lane and engine before the drain.
            tail = out_dmas + [last_mm, last_act, last_dve]
            for d in tail:
                n = nc.sync.nop(nofuse=True)
                add_dep_helper(
                    n.ins, d.ins, sync=True, reason="observe final ticks pre-drain"
                )

    return nc


def _get_nc():
    global _CACHED_NC
    if _CACHED_NC is None:
        _CACHED_NC = _build_nc()
    return _CACHED_NC


def _prep_in_maps(representation, W1, b1, W2, b2):
    rep = np.asarray(representation, dtype=np.float32).reshape(B, R).astype(BF16)
    w1 = np.asarray(W1, dtype=np.float32).astype(BF16)
    w2 = np.asarray(W2, dtype=np.float32).astype(BF16)
    b1 = np.asarray(b1, dtype=np.float32).astype(BF16)
    b2 = np.asarray(b2, dtype=np.float32).astype(BF16)

    w1p = np.ascontiguousarray(
        w1.T.reshape(RC, 128, HID).transpose(1, 0, 2).reshape(128, RC * HID)
    )
    w2p = np.ascontiguousarray(
        w2.T.reshape(HC, 128, OUT).transpose(1, 0, 2).reshape(128, HC * OUT)
    )
    prow = np.zeros((1, PROWW), dtype=BF16)
    prow[0, B1OFF : B1OFF + HID] = b1
    prow[0, ONOFF : ONOFF + 128] = 1.0
    prow[0, B2OFF : B2OFF + OUT] = b2
    wBp = np.ascontiguousarray(w1p[:, RCA * HID :])

    in_maps = []
    for c in range(N_CORES):
        xtc = rep[c * BPC : (c + 1) * BPC].T  # [R, BPC]
        wAp = np.zeros((128, WAW), dtype=BF16)
        wAp[:, XTOFF : XTOFF + RC * BPC] = (
            xtc.reshape(RC, 128, BPC).transpose(1, 0, 2).reshape(128, RC * BPC)
        )
        wAp[0:BPC, I4OFF : I4OFF + BPC] = np.eye(BPC, dtype=BF16)
        for b in range(BPC):
            wAp[b, SELOFF + b * 128 : SELOFF + (b + 1) * 128] = 1.0
        wAp[:, W1AOFF:] = w1p[:, : RCA * HID]
        in_maps.append({"wA": wAp, "wB": wBp, "prow": prow, "w2": w2p})
    return in_maps


def run_sharded(representation, W1, b1, W2, b2, **run_kwargs):
    """Compile+run on 8 cores; returns (full_output, BassKernelResults)."""
    from concourse.bass_utils import run_bass_kernel_spmd

    nc = _get_nc()
    in_maps = _prep_in_maps(representation, W1, b1, W2, b2)
    res = run_bass_kernel_spmd(nc, in_maps, core_ids=list(range(N_CORES)), **run_kwargs)
    full = np.concatenate(
        [np.asarray(r["out"]).astype(np.float32) for r in res.results], axis=0
    )
    return full, res


def kernel(representation, size_matrix=None, W1=None, b1=None, W2=None, b2=None):
    # size_matrix only contributes its shape in the reference (ones_like);
    # its values are unused.
    full, _ = run_sharded(representation, W1, b1, W2, b2)
    return full


# revision 18
# speedup vs baseline: 1.1501x; 1.1224x over previous
"""Trainium2 Bass kernel for nn_Decoder_14894946583396 (dense_mlp).

Reference computation:
    sized = broadcast(representation[B,1,R] -> [B,S,R])   (ones @ rep)
    h     = relu(sized @ W1^T + b1)                       [B,S,HID]
    out   = h @ W2^T + b2                                 [B,S,OUT]

Because every position s within batch b receives the identical input row
representation[b], the MLP output row is identical for all S positions:
    row[b] = relu(rep[b] @ W1^T + b1) @ W2^T + b2         [B,OUT]
    out[b, s, :] = row[b]  for all s

The kernel computes the tiny per-batch MLP on the TensorEngine and
broadcast-writes each row across S with wide SBUF->DRAM DMAs.
Data-parallel across 8 NeuronCores: 4 batches per core, replicated
weights.

Everything on the device runs in bf16 (f32 PSUM accumulation): the
tolerance is 2e-2 and the measured end-to-end bf16 rounding error is
~4e-3.  This halves all three DMA phases vs an f32 version and doubles
PE matmul throughput; the output is stored as bf16 [BPC,S,OUT]
(8 MiB/core) which the host upcasts to f32 after the gather.

Device pipeline per core:
  1. Four input DMAs in sync-ring HWDGE FIFO order:
       wA = {x^T chunks, 4x4 identity, selector blocks, W1^T rc0-3}
       wB = {W1^T rc4-7}
       prow = {b1, ones, b2}        (tiny)
       w2 = {W2^T}
     L1's first half starts as soon as wA's semaphore fires (~10.5 us),
     rc4-7 pipeline behind wB, and w2 lands exactly when L2 needs it.
     (The scalar HWDGE ring only drains after the sync ring's entire
     queue, and SWDGE's SBUF descriptor rings contend with SDMA engines
     7/15, so everything rides the sync ring.)
  2. Dummy matmuls on zeros bridge the PE from kernel start until wA
     lands, warming the HAM clock gate (1.2 -> 2.4 GHz); the last
     warmup's LDWEIGHTS carries the wA-lane wait so L1 itself never
     stalls cold.
  3. L1: H[m,h] = x @ W1^T via 8 accumulating bf16 matmuls with the
     tiny x^T chunk as stationary (cheap LDWEIGHTS), bias folded in as
     a K=1 ones-matmul at the end, relu on ScalarE (PSUM f32 -> SBUF
     bf16).
  4. H -> H^T via 4 PE transposes (stationary operand for L2).
  5. L2: Y[m,o] = H @ W2^T + b2, 10 matmuls into 2 PSUM banks.
  6. Broadcast: K=4 selector matmul (lhsT = e_b outer ones) extracts
     row b of Y AND replicates it across all 128 output partitions in
     one PE op -> [128,512] PSUM tiles; one PSUM->SBUF bf16 copy per
     half, then log-doubling SBUF->SBUF replication copies.  Writer
     engines: b0/b2 -> DVE (with an order-only edge keeping b2 behind
     b0, whose tile gates the first output DMA), b1/b3 -> ACT.
  7. 8 output DMAs of 1 MiB each (bf16) on the sync HWDGE ring: all
     j=0 DMAs first, then the j=1 group — the 12 HWDGE DMAs round-robin
     over 8 DMAHW lanes, so the last four wrap onto the input lanes and
     carry a non-elidable lane-reuse wait; they must be DMAs whose data
     wait is already covered (the j=1 group, covered by their j=0
     siblings).

Single-sync-wait discipline (this walrus rejects 2+ waits on any
instruction): every consumer sees at most one new semaphore because
instructions with free wait slots pre-observe upcoming lanes (last
warmup observes wA, L1 rc4 waits wB naturally, the L1 bias matmul
waits prow naturally, transpose hc=1 observes w2) and vector clocks
propagate transitively through the copy chain, so the output DMAs can
reuse input DMAHW lanes without extra waits.  A chain of 1-wait SP
nops before the TileContext exit drain leaves the drain with nothing
to wait on.
"""

import sys

import numpy as np

if "/opt/trn_rl_repo" not in sys.path:
    sys.path.insert(0, "/opt/trn_rl_repo")

import ml_dtypes

BF16 = np.dtype(ml_dtypes.bfloat16)

B, S, R = 32, 1024, 1024
HID, OUT = 512, 1024
N_CORES = 8
BPC = B // N_CORES  # batches per core

RC = R // 128  # layer-1 contraction chunks
RCA = RC // 2  # chunks carried by wA (rest in wB)
HC = HID // 128  # layer-2 contraction chunks
OC = OUT // 512  # 512-wide output column chunks

# wA columns: x^T chunks [p, rc*BPC + m] = rep[m, rc*128+p], then a
# 512-wide bias band whose rows sit at matmul-legal partition bases
# (p0 = b2 half 1, p32 = b1, p64 = ones, p96 = b2 half 0), then W1^T
# chunks rc0..RCA-1 ([p, W1AOFF + rc*HID + h] = W1[h, rc*128+p]).
XTOFF = 0
BIASOFF = XTOFF + RC * BPC
ONES4 = BIASOFF + HID  # 4-wide ones block shared column-wise by each row
W1AOFF = ONES4 + BPC
WAW = W1AOFF + RCA * HID
# wB columns: 4x4 transpose identity, selector-broadcast blocks
# [k, SELOFF + b*128 + m] = (k == b), then W1^T chunks RCA..RC-1
I4OFF = 0
SELOFF = I4OFF + BPC
W1BOFF = SELOFF + BPC * 128
WBW = W1BOFF + (RC - RCA) * HID
# w2: [p, hc*OUT + o] = W2[o, hc*128+p]

N_COPIES = 4  # row copies along the free dim of each broadcast tile
S_PER_DMA = 128 * N_COPIES  # s-positions covered per output DMA
N_DMAS = S // S_PER_DMA  # output DMAs per batch
N_WARMUP = 7

_CACHED_NC = None


def _build_nc():
    import concourse.bass as bass
    import concourse.mybir as mybir
    from concourse.tile import TileContext, add_dep_helper

    bf16 = mybir.dt.bfloat16
    f32 = mybir.dt.float32
    relu = mybir.ActivationFunctionType.Relu
    fcopy = mybir.ActivationFunctionType.Copy
    nc = bass.Bass()

    wA = nc.dram_tensor("wA", [128, WAW], bf16, kind="ExternalInput")
    wB = nc.dram_tensor("wB", [128, WBW], bf16, kind="ExternalInput")
    w2 = nc.dram_tensor("w2", [128, HC * OUT], bf16, kind="ExternalInput")
    out = nc.dram_tensor("out", [BPC, S, OUT], bf16, kind="ExternalOutput")

    with TileContext(nc) as tc:
        with (
            tc.tile_pool(name="const", bufs=1) as cpool,
            tc.tile_pool(name="psum_s", bufs=1, space="PSUM") as pp_s,
            tc.tile_pool(name="psum_y", bufs=2, space="PSUM") as pp_y,
            tc.tile_pool(name="psum_t", bufs=2, space="PSUM") as pp_t,
            tc.tile_pool(name="psum_bc", bufs=3, space="PSUM") as pp_bc,
        ):
            wA_sb = cpool.tile([128, WAW], bf16, tag="wA")
            dma_wA = nc.sync.dma_start(out=wA_sb[:, :], in_=wA[:, :])
            wB_sb = cpool.tile([128, WBW], bf16, tag="wB")
            dma_wB = nc.sync.dma_start(out=wB_sb[:, :], in_=wB[:, :])
            w2_sb = cpool.tile([128, HC * OUT], bf16, tag="w2")
            dma_w2 = nc.sync.dma_start(out=w2_sb[:, :], in_=w2[:, :])

            # ---- PE warmup on zeros; shares L1's PSUM tile (a slot handoff
            # would emit a non-elidable same-engine wait).  Bridges the PE
            # from kernel start until wA's semaphore fires. ------------------
            wm_sb = cpool.tile([128, 512], bf16, tag="wm")
            nc.vector.memset(wm_sb[:, :], 0.0)
            ph_full = pp_s.tile([128, HID], f32, tag="s")
            for k in range(N_WARMUP):
                wmm = nc.tensor.matmul(
                    ph_full[:, :],
                    lhsT=wm_sb[:, 0:128],
                    rhs=wm_sb[:, :],
                    start=True,
                    stop=True,
                )
            # the last warmup's LDWEIGHTS carries the wA-lane wait: earlier
            # warmups keep the PE busy while it waits, and L1 never stalls
            add_dep_helper(wmm.ins, dma_wA.ins, sync=True, reason="observe wA")

            # ---- L1: H[m, h] = x @ W1^T + b1, relu -------------------------
            ph = ph_full[0:BPC, :]
            for rc in range(RC):
                w1_src = wA_sb if rc < RCA else wB_sb
                w1_off = (
                    W1AOFF + rc * HID if rc < RCA else W1BOFF + (rc - RCA) * HID
                )
                mm = nc.tensor.matmul(
                    ph[:, :],
                    lhsT=wA_sb[:, XTOFF + rc * BPC : XTOFF + (rc + 1) * BPC],
                    rhs=w1_src[:, w1_off : w1_off + HID],
                    start=(rc == 0),
                    stop=False,
                )
            # bias last: a K=1 ones-matmul (ones row at partition 64, b1 row
            # at partition 32 of wA's bias band)
            nc.tensor.matmul(
                ph[:, :],
                lhsT=wA_sb[32:33, ONES4 : ONES4 + BPC],
                rhs=wA_sb[32:33, BIASOFF : BIASOFF + HID],
                start=False,
                stop=True,
            )
            h_sb = cpool.tile([BPC, HID], bf16, tag="h")
            nc.scalar.activation(h_sb[:, :], ph[:, :], relu)

            # ---- H -> H^T (stationary operand for L2) ----------------------
            ht_sb = cpool.tile([128, HC * BPC], bf16, tag="ht")
            for hc in range(HC):
                pt = pp_t.tile([128, BPC], bf16, tag="t")
                tr = nc.tensor.transpose(
                    pt[:, :],
                    h_sb[0:BPC, hc * 128 : (hc + 1) * 128],
                    wB_sb[0:BPC, I4OFF : I4OFF + BPC],
                )
                if hc == 1:
                    # free wait slot (h_sb wait covered by hc=0, fresh PSUM
                    # slot): pre-observe w2's lane for L2 off the critical
                    # path
                    add_dep_helper(tr.ins, dma_w2.ins, sync=True, reason="observe w2")
                nc.scalar.activation(
                    ht_sb[:, hc * BPC : (hc + 1) * BPC], pt[:, :], fcopy
                )

            # ---- L2: Y[m, o] = H @ W2^T + b2 -------------------------------
            # per-oc Y tiles so the broadcast of the first half can start
            # while the second half's matmuls still run
            y_halves = []
            for oc in range(OC):
                py = pp_y.tile([BPC, 512], f32, tag="y")
                for hc in range(HC):
                    nc.tensor.matmul(
                        py[:, :],
                        lhsT=ht_sb[:, hc * BPC : (hc + 1) * BPC],
                        rhs=w2_sb[:, hc * OUT + oc * 512 : hc * OUT + oc * 512 + 512],
                        start=(hc == 0),
                        stop=False,
                    )
                bp = 64 if oc == 0 else 0
                nc.tensor.matmul(
                    py[:, :],
                    lhsT=wA_sb[bp : bp + 1, ONES4 : ONES4 + BPC],
                    rhs=wA_sb[bp : bp + 1, BIASOFF : BIASOFF + 512],
                    start=False,
                    stop=True,
                )
                yh = cpool.tile([BPC, 512], bf16, tag=f"yh{oc}")
                nc.scalar.activation(yh[:, :], py[:, :], fcopy)
                y_halves.append(yh)

            # ---- broadcast rows across partitions, replicate, store --------
            # A K=4 selector matmul (lhsT = e_b outer ones, host-packed)
            # extracts row b of Y AND replicates it across all 128 output
            # partitions in one PE op — both operands at base partition 0.
            out_dmas = []
            yts = []
            for b in range(BPC):
                yt = cpool.tile([128, N_COPIES * OUT], bf16, tag=f"yt{b}")
                copy_eng = "dve" if b % 2 == 0 else "act"
                if b == 2:
                    first_of_b2 = True
                for oc in range(OC):
                    pb = pp_bc.tile([128, 512], f32, tag="bc")
                    mm = nc.tensor.matmul(
                        pb[:, :],
                        lhsT=wB_sb[0:BPC, SELOFF + b * 128 : SELOFF + (b + 1) * 128],
                        rhs=y_halves[oc][0:BPC, :],
                        start=True,
                        stop=True,
                    )
                    last_mm = mm
                    # PSUM -> SBUF once per oc half (f32 -> bf16 cast)...
                    dst = yt[:, oc * 512 : (oc + 1) * 512]
                    if copy_eng == "dve":
                        cp = nc.vector.tensor_copy(dst, pb[:, :])
                        if b == 2 and first_of_b2:
                            # order-only edge: keep b2's DVE work behind b0's
                            # last doubling (b0's tile gates the first DMA)
                            add_dep_helper(
                                cp.ins, last_dve.ins, sync=False,
                                reason="order b2 after b0 doublings",
                            )
                            first_of_b2 = False
                        last_dve = cp
                    else:
                        last_act = nc.scalar.activation(dst, pb[:, :], fcopy)
                # ...then replicate with log-doubling SBUF->SBUF copies
                ncur = OUT
                while ncur < N_COPIES * OUT:
                    dst = yt[:, ncur : 2 * ncur]
                    if copy_eng == "dve":
                        last_dve = nc.vector.tensor_copy(dst, yt[:, 0:ncur])
                    else:
                        last_act = nc.scalar.activation(dst, yt[:, 0:ncur], fcopy)
                    ncur *= 2
                yts.append(yt)

            # Output DMAs: each writes S_PER_DMA consecutive s rows (all
            # identical).  All j=0 DMAs first, then the j=1 group (see
            # docstring).
            for j in range(N_DMAS):
                for b in range(BPC):
                    d = nc.sync.dma_start(
                        out=out[b, j * S_PER_DMA : (j + 1) * S_PER_DMA, :].rearrange(
                            "(p c) o -> p c o", c=N_COPIES
                        ),
                        in_=yts[b][:, :].rearrange("p (c o) -> p c o", o=OUT),
                    )
                    out_dmas.append(d)

            # The kernel-tail drain waits on every proc's final tick, but this
            # walrus allows at most ONE sync wait per instruction. Chain SP
            # nops, one dependency each, so SP's vector clock observes the
            # final tick of every DMA lane and engine before the drain.
            tail = out_dmas + [last_mm, last_act, last_dve]
            for d in tail:
                n = nc.sync.nop(nofuse=True)
                add_dep_helper(
                    n.ins, d.ins, sync=True, reason="observe final ticks pre-drain"
                )

    return nc


def _get_nc():
    global _CACHED_NC
    if _CACHED_NC is None:
        _CACHED_NC = _build_nc()
    return _CACHED_NC


def _prep_in_maps(representation, W1, b1, W2, b2):
    rep = np.asarray(representation, dtype=np.float32).reshape(B, R).astype(BF16)
    w1 = np.asarray(W1, dtype=np.float32).astype(BF16)
    w2 = np.asarray(W2, dtype=np.float32).astype(BF16)
    b1 = np.asarray(b1, dtype=np.float32).astype(BF16)
    b2 = np.asarray(b2, dtype=np.float32).astype(BF16)

    w1p = np.ascontiguousarray(
        w1.T.reshape(RC, 128, HID).transpose(1, 0, 2).reshape(128, RC * HID)
    )
    w2p = np.ascontiguousarray(
        w2.T.reshape(HC, 128, OUT).transpose(1, 0, 2).reshape(128, HC * OUT)
    )
    wBp = np.zeros((128, WBW), dtype=BF16)
    wBp[0:BPC, I4OFF : I4OFF + BPC] = np.eye(BPC, dtype=BF16)
    for b in range(BPC):
        wBp[b, SELOFF + b * 128 : SELOFF + (b + 1) * 128] = 1.0
    wBp[:, W1BOFF:] = w1p[:, RCA * HID :]

    in_maps = []
    for c in range(N_CORES):
        xtc = rep[c * BPC : (c + 1) * BPC].T  # [R, BPC]
        wAp = np.zeros((128, WAW), dtype=BF16)
        wAp[:, XTOFF : XTOFF + RC * BPC] = (
            xtc.reshape(RC, 128, BPC).transpose(1, 0, 2).reshape(128, RC * BPC)
        )
        # bias band: rows at matmul-legal partition bases, each with its
        # own 4-wide ones block at the same partition (lhsT and rhs of a
        # matmul must share a base partition)
        wAp[0, BIASOFF : BIASOFF + 512] = b2[512:]
        wAp[32, BIASOFF : BIASOFF + HID] = b1
        wAp[64, BIASOFF : BIASOFF + 512] = b2[:512]
        wAp[0, ONES4 : ONES4 + BPC] = 1.0
        wAp[32, ONES4 : ONES4 + BPC] = 1.0
        wAp[64, ONES4 : ONES4 + BPC] = 1.0
        wAp[:, W1AOFF:] = w1p[:, : RCA * HID]
        in_maps.append({"wA": wAp, "wB": wBp, "w2": w2p})
    return in_maps


def run_sharded(representation, W1, b1, W2, b2, **run_kwargs):
    """Compile+run on 8 cores; returns (full_output, BassKernelResults)."""
    from concourse.bass_utils import run_bass_kernel_spmd

    nc = _get_nc()
    in_maps = _prep_in_maps(representation, W1, b1, W2, b2)
    res = run_bass_kernel_spmd(nc, in_maps, core_ids=list(range(N_CORES)), **run_kwargs)
    full = np.concatenate(
        [np.asarray(r["out"]).astype(np.float32) for r in res.results], axis=0
    )
    return full, res


def kernel(representation, size_matrix=None, W1=None, b1=None, W2=None, b2=None):
    # size_matrix only contributes its shape in the reference (ones_like);
    # its values are unused.
    full, _ = run_sharded(representation, W1, b1, W2, b2)
    return full
